# revision 1
# baseline (speedup 1.0000x reference)
"""GNN edge-scorer kernel for Trainium2 (8 NeuronCores, SPMD).

reference:
    edge_emb = concat(emb[src], emb[dst])          # [E, 128]
    h = relu(edge_emb @ W1 + b1)                   # [E, 64]
    logits = h @ W2 + b2                           # [E, 1]

Device algorithm (memory-bound gather regime):
  Phase A (per core, replicated): AB[n] = [emb[n] @ W1[:64] + b1 | emb[n] @ W1[64:]]
    via PE matmul with K=65 (ones-row folds b1).  AB is [150016, 128] f32 in DRAM,
    row = 512 B.
  Phase B: per edge, gather A-half of AB[src] (256 B) and B-half of AB[dst]
    (256 B) with dma_gather (int16 idx, elem_step=128), then on DVE/ACT:
    h = relu(ga + gb); logit = sum(h * w2_rep) + b2.

  dma_gather int16 indices only span 32768 rows -> table split into 5 chunks;
  host sorts edges into 25 (src_chunk, dst_chunk) buckets and deals them
  round-robin over the 8 cores so every core has identical bucket sizes
  (single SPMD program).  Output unscrambled on host.
"""

import numpy as np

N_NODES = 150000
TRACE = False            # set by test harness to capture HW profile
_last_results = None     # BassKernelResults of the most recent run
_last_ctx = None         # (nc, in_maps) of the most recent run
_last_names = None
_last_s_b = None
D = 64
P = 128
N_CORES = 8
CHUNK = 32768                      # dma_gather int16 index range
NT = 1024                          # precompute nodes per tile
MAXG = 8192                        # max indices per dma_gather instruction
QUEUES = 4                         # SWDGE queues for gather desc-gen


def _derived():
    n_chunks = (N_NODES + CHUNK - 1) // CHUNK
    r_pad = 128 * ((N_NODES + 127) // 128)
    return n_chunks, r_pad


def _round_up(x, m):
    return (x + m - 1) // m * m


def _build_program(s_b, loop_n=None, mode="full", queues=1):
    """Build the SPMD Bass program.

    s_b: list of 25 per-bucket slot counts (slots of 128 edges, same on
    every core).  Returns (nc, names dict).
    """
    import concourse.bacc as bacc
    import concourse.mybir as mybir
    import concourse.tile as tile

    N_CHUNKS, R_PAD = _derived()
    tot_slots = int(sum(s_b))
    tot_idx = tot_slots * P

    nc = bacc.Bacc(None, target_bir_lowering=False, debug=False, num_swdge_queues=queues)
    with tile.TileContext(nc) as tc:
        with tc.tile_pool(name="dram", bufs=1, space="DRAM") as dram:
            embT_t = dram.tile([D + 1, R_PAD], mybir.dt.float32, kind="ExternalInput")
            w1cat_t = dram.tile([D + 1, 2 * D], mybir.dt.float32, kind="ExternalInput")
            w2rep_t = dram.tile([P, D], mybir.dt.float32, kind="ExternalInput")
            b2b_t = dram.tile([P, 1], mybir.dt.float32, kind="ExternalInput")
            isrc_t = dram.tile([P, tot_idx // 16], mybir.dt.int16, kind="ExternalInput")
            idst_t = dram.tile([P, tot_idx // 16], mybir.dt.int16, kind="ExternalInput")
            out_t = dram.tile([P, tot_slots], mybir.dt.float32, kind="ExternalOutput")

            # AB table, one DRAM tile per 32768-row chunk so Tile can
            # pipeline gathers of chunk c behind the writes of chunk c.
            ab_rows = [CHUNK] * (N_CHUNKS - 1) + [R_PAD - CHUNK * (N_CHUNKS - 1)]
            ab_ts = [
                dram.tile([r, 2 * D], mybir.dt.float32, name=f"ab{c}")
                for c, r in enumerate(ab_rows)
            ]

            with (
                tc.tile_pool(name="consts", bufs=1) as consts,
                tc.tile_pool(name="pre_et", bufs=3) as pre_et,
                tc.tile_pool(name="pre_ab", bufs=3) as pre_ab,
                tc.tile_pool(name="ps_ab", bufs=2, space="PSUM") as ps_ab,
                tc.tile_pool(name="idx", bufs=3) as idxp,
                tc.tile_pool(name="ga", bufs=3) as gap,
                tc.tile_pool(name="lg", bufs=3) as lgp,
            ):
                w1cat_sb = consts.tile([D + 1, 2 * D], mybir.dt.float32)
                nc.sync.dma_start(w1cat_sb[:], w1cat_t[:])
                b2b_sb = consts.tile([P, 1], mybir.dt.float32)
                nc.sync.dma_start(b2b_sb[:], b2b_t[:])
                # w2 replicated across MAXG//128 slots -> [P, (MAXG//P)*D]
                w2in_sb = consts.tile([P, D], mybir.dt.float32, tag="w2in")
                nc.sync.dma_start(w2in_sb[:], w2rep_t[:])
                w2rep_sb = consts.tile([P, (MAXG // P) * D], mybir.dt.float32, tag="w2rep")
                for j in range(MAXG // P):
                    nc.vector.tensor_copy(w2rep_sb[:, j * D:(j + 1) * D], w2in_sb[:])

                # ---- interleaved emission: precompute chunks + buckets ----
                def precompute_chunk(c):
                    base = c * CHUNK
                    rows = ab_rows[c]
                    for nt0 in range(0, rows, NT):
                        n = min(NT, rows - nt0)
                        et = pre_et.tile([D + 1, NT], mybir.dt.float32, tag="et")
                        nc.sync.dma_start(
                            et[:, :n], embT_t[:, base + nt0: base + nt0 + n]
                        )
                        ab_sb = pre_ab.tile([P, NT // P * 2 * D], mybir.dt.float32, tag="absb")
                        for k in range(n // P):
                            ab_ps = ps_ab.tile([P, 512], mybir.dt.float32, space="PSUM")
                            nc.tensor.matmul(
                                ab_ps[:, :2 * D],
                                lhsT=et[:, k * P:(k + 1) * P],
                                rhs=w1cat_sb[:],
                                start=True, stop=True,
                            )
                            nc.scalar.activation(
                                ab_sb[:, k * 2 * D:(k + 1) * 2 * D],
                                ab_ps[:, :2 * D],
                                mybir.ActivationFunctionType.Copy,
                            )
                        # AB rows nt0+k*128+p , feat f  <- ab_sb[p, k*128+f]
                        dst = ab_ts[c][nt0:nt0 + n, :].rearrange(
                            "(k p) f -> p k f", p=P
                        )
                        src_ap = ab_sb[:, :n // P * 2 * D].rearrange(
                            "p (k f) -> p k f", f=2 * D
                        )
                        nc.sync.dma_start(dst, src_ap)

                # bucket schedule: emit bucket (cs,cd) after chunks cs, cd
                emitted_chunks = set()
                idx_off = 0   # in int16 columns (tot_idx//16 total)
                out_off = 0   # in slots

                gq = [0]

                def emit_bucket(b):
                    nonlocal idx_off, out_off
                    cs, cd = b // N_CHUNKS, b % N_CHUNKS
                    slots = s_b[b]
                    done = 0
                    while done < slots:
                        g = min(MAXG // P, slots - done)      # slots this gather
                        n_idx = g * P
                        isrc = idxp.tile([P, MAXG // 16], mybir.dt.int16, tag="isrc")
                        idst = idxp.tile([P, MAXG // 16], mybir.dt.int16, tag="idst")
                        nc.sync.dma_start(
                            isrc[:, :n_idx // 16],
                            isrc_t[:, idx_off: idx_off + n_idx // 16],
                        )
                        nc.sync.dma_start(
                            idst[:, :n_idx // 16],
                            idst_t[:, idx_off: idx_off + n_idx // 16],
                        )
                        ga = gap.tile([P, MAXG // P, D], mybir.dt.float32, tag="ga")
                        gb = gap.tile([P, MAXG // P, D], mybir.dt.float32, tag="gb")
                        if mode != "B_dve":
                            nc.gpsimd.dma_gather(
                                ga[:, :g, :], ab_ts[cs][:, 0:D], isrc[:, :n_idx // 16],
                                n_idx, n_idx, D, elem_step=2 * D, single_packet=False,
                                queue_num=gq[0],
                            )
                            gq[0] = (gq[0] + 1) % queues
                            nc.gpsimd.dma_gather(
                                gb[:, :g, :], ab_ts[cd][:, D:2 * D], idst[:, :n_idx // 16],
                                n_idx, n_idx, D, elem_step=2 * D, single_packet=False,
                                queue_num=gq[0],
                            )
                            gq[0] = (gq[0] + 1) % queues
                        # h = relu(ga + gb); logit = sum(h*w2) + b2
                        if mode == "B_gather":
                            idx_off += n_idx // 16
                            out_off += g
                            done += g
                            continue
                        nc.vector.tensor_add(
                            ga[:, :g, :], ga[:, :g, :], gb[:, :g, :]
                        )
                        nc.scalar.activation(
                            ga[:, :g, :], ga[:, :g, :],
                            mybir.ActivationFunctionType.Relu,
                        )
                        nc.vector.tensor_mul(
                            ga[:, :g, :], ga[:, :g, :],
                            w2rep_sb[:, :g * D].rearrange("p (s d) -> p s d", d=D),
                        )
                        lg = lgp.tile([P, MAXG // P], mybir.dt.float32, tag="lg")
                        nc.vector.tensor_reduce(
                            lg[:, :g], ga[:, :g, :],
                            axis=mybir.AxisListType.X, op=mybir.AluOpType.add,
                        )
                        nc.vector.tensor_scalar_add(
                            lg[:, :g], lg[:, :g], b2b_sb[:, :1]
                        )
                        nc.sync.dma_start(
                            out_t[:, out_off: out_off + g], lg[:, :g]
                        )
                        idx_off += n_idx // 16
                        out_off += g
                        done += g

                # schedule: chunks in order; after chunk c, all buckets whose
                # max(cs, cd) == c
                def emit_all():
                    nonlocal idx_off, out_off
                    idx_off = 0
                    out_off = 0
                    for c in range(N_CHUNKS):
                        precompute_chunk(c)
                        emitted_chunks.add(c)
                        for b in range(N_CHUNKS * N_CHUNKS):
                            cs, cd = b // N_CHUNKS, b % N_CHUNKS
                            if max(cs, cd) == c and s_b[b] > 0:
                                emit_bucket(b)

                def emit_buckets_only():
                    nonlocal idx_off, out_off
                    idx_off = 0
                    out_off = 0
                    for c in range(N_CHUNKS):
                        for b in range(N_CHUNKS * N_CHUNKS):
                            cs, cd = b // N_CHUNKS, b % N_CHUNKS
                            if max(cs, cd) == c and s_b[b] > 0:
                                emit_bucket(b)

                if loop_n is None:
                    emit_all()
                elif mode == "full":
                    with tc.For_i(0, loop_n, 1):
                        emit_all()
                else:
                    for c in range(N_CHUNKS):
                        precompute_chunk(c)
                    with tc.For_i(0, loop_n, 1):
                        emit_buckets_only()

    nc.compile()
    names = dict(
        embT=embT_t.name, w1cat=w1cat_t.name, w2rep=w2rep_t.name,
        b2b=b2b_t.name, isrc=isrc_t.name, idst=idst_t.name, out=out_t.name,
    )
    return nc, names, tot_slots


def kernel(nodes_emb, src, dst, W1, b1, W2, b2):
    from concourse.bass_utils import run_bass_kernel_spmd

    nodes_emb = np.ascontiguousarray(np.asarray(nodes_emb, dtype=np.float32))
    src = np.asarray(src).astype(np.int64)
    dst = np.asarray(dst).astype(np.int64)
    W1 = np.asarray(W1, dtype=np.float32)
    b1 = np.asarray(b1, dtype=np.float32).reshape(-1)
    W2 = np.asarray(W2, dtype=np.float32)
    b2 = np.asarray(b2, dtype=np.float32).reshape(-1)
    E = src.shape[0]
    N_CHUNKS, R_PAD = _derived()

    # ---- host prep -------------------------------------------------------
    # embT65 [65, R_PAD]: emb^T padded with zeros, plus a ones row (bias)
    embT = np.zeros((D + 1, R_PAD), dtype=np.float32)
    embT[:D, :N_NODES] = nodes_emb.T
    embT[D, :] = 1.0
    # w1cat65 [65, 128]: cols 0:64 -> A-half (W1[:64] with b1), 64:128 -> B-half
    w1cat = np.zeros((D + 1, 2 * D), dtype=np.float32)
    w1cat[:D, :D] = W1[:D]
    w1cat[:D, D:] = W1[D:]
    w1cat[D, :D] = b1            # bias folded into A-half
    w2rep = np.tile(W2.reshape(1, D), (P, 1)).astype(np.float32)
    b2b = np.full((P, 1), b2[0], dtype=np.float32)

    # ---- bucket sort + deal over cores ----------------------------------
    cs = src // CHUNK
    cd = dst // CHUNK
    bucket = (cs * N_CHUNKS + cd).astype(np.int64)
    order = np.argsort(bucket, kind="stable")          # edge ids, bucket-major
    bcounts = np.bincount(bucket, minlength=N_CHUNKS * N_CHUNKS)
    # per-core-per-bucket count (round-robin deal), padded to 128
    m_bc = -(-bcounts // N_CORES)                      # ceil
    s_b = [int(_round_up(m, P) // P) if m > 0 else 0 for m in m_bc]

    nc, names, tot_slots = _build_program(s_b, queues=QUEUES)
    global _last_names, _last_s_b
    _last_names = names
    _last_s_b = s_b
    tot_idx = tot_slots * P

    src_s = (src[order] % CHUNK).astype(np.int16)
    dst_s = (dst[order] % CHUNK).astype(np.int16)

    in_maps = []
    # host index bookkeeping: for each sorted position, compute its
    # (core, flat device stream index)
    core_of = np.empty(E, dtype=np.int64)
    stream_of = np.empty(E, dtype=np.int64)
    bstart = np.concatenate([[0], np.cumsum(bcounts)])
    # device consumes buckets in emit order: bucket (cs, cd) is emitted after
    # AB chunks cs and cd, i.e. grouped by max(cs, cd)
    emit_order = [
        b for c in range(N_CHUNKS)
        for b in range(N_CHUNKS * N_CHUNKS)
        if max(b // N_CHUNKS, b % N_CHUNKS) == c
    ]
    slot_off = np.zeros(N_CHUNKS * N_CHUNKS, dtype=np.int64)
    acc = 0
    for b in emit_order:
        slot_off[b] = acc
        acc += s_b[b]
    for b in range(N_CHUNKS * N_CHUNKS):
        nb = bcounts[b]
        if nb == 0:
            continue
        pos = np.arange(nb)
        core_of[bstart[b]: bstart[b + 1]] = pos % N_CORES
        stream_of[bstart[b]: bstart[b + 1]] = slot_off[b] * P + pos // N_CORES

    isrc_all = np.zeros((N_CORES, tot_idx), dtype=np.int16)
    idst_all = np.zeros((N_CORES, tot_idx), dtype=np.int16)
    for c in range(N_CORES):
        m = core_of == c
        isrc_all[c, stream_of[m]] = src_s[m]
        idst_all[c, stream_of[m]] = dst_s[m]

    def wrap16(a):
        # stream index i -> [i % 16, i // 16], replicated to 128 partitions
        w = a.reshape(-1, 16).T                        # [16, tot/16]
        return np.tile(w, (8, 1)).copy()

    for c in range(N_CORES):
        in_maps.append({
            names["embT"]: embT,
            names["w1cat"]: w1cat,
            names["w2rep"]: w2rep,
            names["b2b"]: b2b,
            names["isrc"]: wrap16(isrc_all[c]),
            names["idst"]: wrap16(idst_all[c]),
        })

    global _last_results, _last_ctx
    res = run_bass_kernel_spmd(
        nc, in_maps, core_ids=list(range(N_CORES)), trace=TRACE,
    )
    _last_results = res
    _last_ctx = (nc, in_maps)

    # ---- unscramble ------------------------------------------------------
    # device out [128, tot_slots]: stream index i -> out[i % 128, i // 128]
    logits_sorted = np.empty(E, dtype=np.float32)
    for c in range(N_CORES):
        o = res.results[c][names["out"]]               # [128, tot_slots]
        m = core_of == c
        si = stream_of[m]
        logits_sorted[np.flatnonzero(m)] = o[si % P, si // P]
    out = np.empty(E, dtype=np.float32)
    out[order] = logits_sorted
    return out.reshape(E, 1)


def bench(n_iters=16, n_warmup=3):
    """Re-execute the last-compiled SPMD program on device-resident inputs.

    Returns list of per-iteration wall seconds (device exec + dispatch).
    """
    import time
    import jax
    import numpy as np
    from jax.sharding import Mesh, PartitionSpec
    from jax.experimental.shard_map import shard_map
    import concourse.mybir as mybir
    from concourse import bass2jax

    nc, in_maps = _last_ctx
    n_cores = len(in_maps)
    partition_name = nc.partition_id_tensor.name if nc.partition_id_tensor else None

    in_names, out_names, out_avals, zero_outs = [], [], [], []
    for alloc in nc.m.functions[0].allocations:
        if not isinstance(alloc, mybir.MemoryLocationSet):
            continue
        name = alloc.memorylocations[0].name
        if alloc.kind == "ExternalInput":
            if name != partition_name:
                in_names.append(name)
        elif alloc.kind == "ExternalOutput":
            shape = tuple(alloc.tensor_shape)
            dtype = mybir.dt.np(alloc.dtype)
            out_names.append(name)
            out_avals.append(jax.core.ShapedArray(shape, dtype))
            zero_outs.append(np.zeros(shape, dtype))
    n_params = len(in_names)
    n_outs = len(out_avals)
    in_names_all = in_names + out_names
    if partition_name is not None:
        in_names_all = in_names_all + [partition_name]

    def _body(*args):
        operands = list(args)
        if partition_name is not None:
            operands.append(bass2jax.partition_id_tensor())
        outs = bass2jax._bass_exec_p.bind(
            *operands,
            out_avals=tuple(out_avals),
            in_names=tuple(in_names_all),
            out_names=tuple(out_names),
            lowering_input_output_aliases=(),
            sim_require_finite=True,
            sim_require_nnan=True,
            nc=nc,
        )
        return tuple(outs)

    devices = jax.devices()[:n_cores]
    mesh = Mesh(np.asarray(devices), ("core",))
    in_specs = (PartitionSpec("core"),) * (n_params + n_outs)
    out_specs = (PartitionSpec("core"),) * n_outs
    donate = tuple(range(n_params, n_params + n_outs))
    sharded = jax.jit(
        shard_map(_body, mesh=mesh, in_specs=in_specs, out_specs=out_specs,
                  check_rep=False),
        donate_argnums=donate, keep_unused=True,
    )
    sharding = jax.sharding.NamedSharding(mesh, PartitionSpec("core"))
    concat_in = [
        jax.device_put(
            np.concatenate([np.asarray(in_maps[c][name]) for c in range(n_cores)], axis=0),
            sharding,
        )
        for name in in_names
    ]
    jax.block_until_ready(concat_in)
    n_total = n_warmup + n_iters
    zero_sets = [
        [
            jax.device_put(
                np.zeros((n_cores * z.shape[0], *z.shape[1:]), z.dtype), sharding
            )
            for z in zero_outs
        ]
        for _ in range(n_total)
    ]
    jax.block_until_ready(zero_sets)

    times = []
    for i in range(n_total):
        t0 = time.perf_counter()
        out = sharded(*concat_in, *zero_sets[i])
        jax.block_until_ready(out)
        times.append(time.perf_counter() - t0)
        del out
    return times[n_warmup:]



# revision 2
# speedup vs baseline: 2.3367x; 2.3367x over previous
"""GNN edge-scorer for Trainium2 (8 NeuronCores, SPMD) — hybrid gather.

reference:
    edge_emb = concat(emb[src], emb[dst])          # [E, 128]
    h = relu(edge_emb @ W1 + b1)                   # [E, 64]
    logits = h @ W2 + b2                           # [E, 1]

Decomposition: A[n] = emb[n] @ W1[:64] + b1,  B[n] = emb[n] @ W1[64:]
    logit[e] = relu(A[src_e] + B[dst_e]) . w2 + b2

Distribution: core c owns dst-bucket [c*18752, (c+1)*18752); its edges are
sorted by src.  The per-core B-table (18944 rows f32, node-major in DRAM)
is computed on-device from a per-core embT slice; dst indices are
bucket-local int16 and one dma_gather source serves the whole run (no chunk
switching, 1 descriptor per edge instead of the baseline's 2).

The A-side uses no DMA gather at all.  Sorted-by-src edges form ~200-edge
runs per 128-node group ("cells").  Per 128-edge PSUM group, one matmul per
intersecting cell accumulates the A rows into PSUM: stationary = a one-hot
[node-low x edge] built on DVE (is_equal of PE-broadcast src ids against
per-group iota columns), moving = a 64-node-group bf16 A-table window that
phase A materializes directly in SBUF (no DRAM roundtrip), just-in-time in
source-group order.  Cell sizes are padded to the max across cores so one
SPMD program fits all cores; the host unscrambles the output.

Per-edge chain: h = relu(ps_A + gb) on DVE/ACT, then *w2rep, reduce, +b2.
"""

import numpy as np

N_NODES = 150000
TRACE = False
_last_results = None
_last_ctx = None
_last_names = None
_last_sched = None
D = 64
P = 128
N_CORES = 8
R_PAD = 150016                  # 293 * 512
B_NODES = R_PAD // N_CORES      # 18752 dst rows owned per core
B_PAD = 18944                   # 37 * 512
PRE_T = 512                     # phase-A nodes per tile
MAXG = 2048                     # edges per phase-B tile (16 PSUM col groups)
W_NODES = 8192                  # A-window nodes (64 groups of 128)
N_GROUPS = R_PAD // P           # 1172 src groups
QUEUES = 4


def _build_schedule(m_g):
    """Static tile schedule from per-src-group padded counts.

    Returns (tot_idx, tiles).  tiles: list of dicts
      nt       edges in tile (multiple of 128, <= MAXG)
      windows  A-windows to fill before this tile
      nranks   max visits over the tile's groups
      visits   [(j_local, r, g_or_None, start, stop)]
    """
    cells = [(g, int(m)) for g, m in enumerate(m_g) if m > 0]
    tot = int(sum(m for _, m in cells))
    tot_pad = -(-tot // P) * P
    if tot_pad > tot:
        cells.append((None, tot_pad - tot))
    n_groups_tot = tot_pad // P
    group_visits = [[] for _ in range(n_groups_tot)]
    pos = 0
    for g, m in cells:
        if g is not None:
            for j in range(pos // P, (pos + m - 1) // P + 1):
                group_visits[j].append(g)
        pos += m
    for gv in group_visits:
        if not gv:
            gv.append(None)

    tiles = []
    seen_w = set()
    j = 0
    while j < n_groups_tot:
        jn = min(16, n_groups_tot - j)
        t = {'nt': jn * P, 'windows': [], 'visits': [],
             'nranks': max(len(group_visits[j + jl]) for jl in range(jn))}
        for jl in range(jn):
            gv = group_visits[j + jl]
            for r, g in enumerate(gv):
                t['visits'].append((jl, r, g, r == 0, r == len(gv) - 1))
                if g is not None:
                    w = (g * P) // W_NODES
                    if w not in seen_w:
                        seen_w.add(w)
                        t['windows'].append(w)
        tiles.append(t)
        j += jn
    return tot_pad, tiles


def _build_program(tiles, tot_idx, loop_n=None, mode="full"):
    import concourse.bacc as bacc
    import concourse.mybir as mybir
    import concourse.tile as tile

    nr_max = max(t['nranks'] for t in tiles)
    gsel_cols = sum(t['nranks'] * 16 for t in tiles)

    nc = bacc.Bacc(None, target_bir_lowering=False, debug=False,
                   num_swdge_queues=QUEUES)
    with tile.TileContext(nc) as tc:
        with tc.tile_pool(name="dram", bufs=1, space="DRAM") as dram:
            embT_t = dram.tile([D + 1, R_PAD], mybir.dt.float32, kind="ExternalInput")
            embTb_t = dram.tile([D + 1, B_PAD], mybir.dt.float32, kind="ExternalInput")
            w1cat_t = dram.tile([D + 1, 2 * D], mybir.dt.float32, kind="ExternalInput")
            w2rep_t = dram.tile([P, D], mybir.dt.float32, kind="ExternalInput")
            b2b_t = dram.tile([P, 1], mybir.dt.float32, kind="ExternalInput")
            ones_t = dram.tile([1, P], mybir.dt.float32, kind="ExternalInput")
            gsel_t = dram.tile([P, gsel_cols], mybir.dt.float32, kind="ExternalInput")
            idst_t = dram.tile([P, tot_idx // 16], mybir.dt.int16, kind="ExternalInput")
            srcf_t = dram.tile([1, tot_idx], mybir.dt.float32, kind="ExternalInput")
            out_t = dram.tile([P, tot_idx // P], mybir.dt.float32, kind="ExternalOutput")
            bt_t = dram.tile([B_PAD, D], mybir.dt.float32, name="btab")

            with (
                tc.tile_pool(name="consts", bufs=1) as consts,
                tc.tile_pool(name="pre_et", bufs=3) as pre_et,
                tc.tile_pool(name="pre_st", bufs=3) as pre_st,
                tc.tile_pool(name="ps_sh", bufs=3, space="PSUM") as ps_sh,
                tc.tile_pool(name="atwp", bufs=3) as atwp,
                tc.tile_pool(name="idxp", bufs=3) as idxp,
                tc.tile_pool(name="srcp", bufs=3) as srcp,
                tc.tile_pool(name="gselp", bufs=3) as gselp,
                tc.tile_pool(name="gbp", bufs=3) as gbp,
                tc.tile_pool(name="ohp", bufs=2) as ohp,
                tc.tile_pool(name="ps_a", bufs=2, space="PSUM") as ps_ap,
                tc.tile_pool(name="lgp", bufs=3) as lgp,
            ):
                w1cat_sb = consts.tile([D + 1, 2 * D], mybir.dt.float32)
                nc.sync.dma_start(w1cat_sb[:], w1cat_t[:])
                b2b_sb = consts.tile([P, 1], mybir.dt.float32)
                nc.sync.dma_start(b2b_sb[:], b2b_t[:])
                ones_sb = consts.tile([1, P], mybir.dt.float32)
                nc.sync.dma_start(ones_sb[:], ones_t[:])
                zero64 = consts.tile([P, D], mybir.dt.bfloat16)
                nc.vector.memset(zero64[:], 0.0)
                w2in_sb = consts.tile([P, D], mybir.dt.float32, tag="w2in")
                nc.sync.dma_start(w2in_sb[:], w2rep_t[:])
                w2rep_sb = consts.tile([P, (MAXG // P) * D], mybir.dt.float32,
                                       tag="w2rep")
                for j in range(MAXG // P):
                    nc.vector.tensor_copy(w2rep_sb[:, j * D:(j + 1) * D],
                                          w2in_sb[:])

                def phase_a_b():
                    # per-core B table (node-major f32) -> bt_t
                    for i in range(B_PAD // PRE_T):
                        et = pre_et.tile([D + 1, PRE_T], mybir.dt.float32, tag="et")
                        nc.sync.dma_start(
                            et[:], embTb_t[:, i * PRE_T:(i + 1) * PRE_T])
                        nstg = pre_st.tile([P, PRE_T // P, D], mybir.dt.float32,
                                           tag="stgb")
                        for q in range(PRE_T // P):
                            ps_n = ps_sh.tile([P, D], mybir.dt.float32,
                                              space="PSUM")
                            nc.tensor.matmul(
                                ps_n[:], lhsT=et[:, q * P:(q + 1) * P],
                                rhs=w1cat_sb[:, D:2 * D], start=True, stop=True)
                            nc.scalar.activation(
                                nstg[:, q, :], ps_n[:],
                                mybir.ActivationFunctionType.Copy)
                        nc.sync.dma_start(
                            bt_t[i * PRE_T:(i + 1) * PRE_T, :].rearrange(
                                "(k p) f -> p k f", p=P),
                            nstg[:])

                atw_tiles = {}

                def fill_window(w):
                    # A-table window (bf16, node-major groups) built in SBUF
                    rows = min(W_NODES, R_PAD - w * W_NODES)
                    atw = atwp.tile([P, (W_NODES // P) * D], mybir.dt.bfloat16,
                                    tag="atw")
                    for i0 in range(0, rows, PRE_T):
                        base = w * W_NODES + i0
                        et = pre_et.tile([D + 1, PRE_T], mybir.dt.float32,
                                         tag="et")
                        nc.sync.dma_start(et[:], embT_t[:, base:base + PRE_T])
                        for q in range(PRE_T // P):
                            ps_n = ps_sh.tile([P, D], mybir.dt.float32,
                                              space="PSUM")
                            nc.tensor.matmul(
                                ps_n[:], lhsT=et[:, q * P:(q + 1) * P],
                                rhs=w1cat_sb[:, 0:D], start=True, stop=True)
                            col = ((i0 + q * P) // P) * D
                            nc.scalar.activation(
                                atw[:, col:col + D], ps_n[:],
                                mybir.ActivationFunctionType.Copy)
                    atw_tiles[w] = atw

                gq = [0]

                def phase_b():
                    idx_off = 0   # int16 idx columns consumed
                    out_off = 0
                    gsel_off = 0
                    for t in tiles:
                        nt = t['nt']
                        ng = nt // P
                        nr = t['nranks']
                        for w in t['windows']:
                            fill_window(w)
                        it = idxp.tile([P, MAXG // 16], mybir.dt.int16, tag="it")
                        nc.sync.dma_start(
                            it[:, :nt // 16],
                            idst_t[:, idx_off:idx_off + nt // 16])
                        gb = gbp.tile([P, MAXG // P, D], mybir.dt.float32,
                                      tag="gb")
                        if mode != "B_noB":
                            nc.gpsimd.dma_gather(
                                gb[:, :ng, :], bt_t[:, :],
                                it[:, :nt // 16], nt, nt, D, elem_step=D,
                                single_packet=False, queue_num=gq[0])
                            gq[0] = (gq[0] + 1) % QUEUES
                        if mode == "B_gather":
                            idx_off += nt // 16
                            out_off += ng
                            continue
                        use_a = mode != "B_noA"
                        if use_a:
                            sf = srcp.tile([1, MAXG], mybir.dt.float32, tag="sf")
                            nc.sync.dma_start(
                                sf[:, :nt],
                                srcf_t[:, idx_off * 16:idx_off * 16 + nt])
                            gs = gselp.tile([P, nr_max * 16], mybir.dt.float32,
                                            tag="gs")
                            nc.sync.dma_start(
                                gs[:, :nr * ng],
                                gsel_t[:, gsel_off:gsel_off + nr * ng])
                            oh = ohp.tile([P, nr_max * MAXG], mybir.dt.bfloat16,
                                          tag="oh")
                            for q0 in range(0, nt, 512):
                                qn = min(512, nt - q0)
                                ps_b = ps_sh.tile([P, 512], mybir.dt.float32,
                                                  space="PSUM")
                                nc.tensor.matmul(
                                    ps_b[:, :qn], lhsT=ones_sb[:],
                                    rhs=sf[:, q0:q0 + qn], start=True, stop=True)
                                for r in range(nr):
                                    nc.vector.tensor_tensor(
                                        oh[:, r * MAXG + q0:
                                           r * MAXG + q0 + qn].rearrange(
                                            "p (j e) -> p j e", e=P),
                                        ps_b[:, :qn].rearrange(
                                            "p (j e) -> p j e", e=P),
                                        gs[:, r * ng + q0 // P:
                                           r * ng + q0 // P + qn // P, None
                                           ].to_broadcast((P, qn // P, P)),
                                        mybir.AluOpType.is_equal)
                            ps_a = ps_ap.tile([P, (MAXG // P) * D],
                                              mybir.dt.float32, space="PSUM")
                            for (jl, r, g, st, sp) in t['visits']:
                                if g is None:
                                    rhs = zero64[:]
                                else:
                                    w = (g * P) // W_NODES
                                    o = ((g * P) % W_NODES // P) * D
                                    rhs = atw_tiles[w][:, o:o + D]
                                nc.tensor.matmul(
                                    ps_a[:, jl * D:(jl + 1) * D],
                                    lhsT=oh[:, r * MAXG + jl * P:
                                            r * MAXG + (jl + 1) * P],
                                    rhs=rhs, start=st, stop=sp)
                        # h = relu(A + B); logits = h . w2 + b2
                        if use_a and mode != "B_noB":
                            nc.vector.tensor_add(
                                gb[:, :ng, :],
                                ps_a[:, :ng * D].rearrange(
                                    "p (k f) -> p k f", f=D),
                                gb[:, :ng, :])
                        elif use_a:
                            nc.vector.tensor_copy(
                                gb[:, :ng, :],
                                ps_a[:, :ng * D].rearrange(
                                    "p (k f) -> p k f", f=D))
                        nc.scalar.activation(
                            gb[:, :ng, :], gb[:, :ng, :],
                            mybir.ActivationFunctionType.Relu)
                        nc.vector.tensor_mul(
                            gb[:, :ng, :], gb[:, :ng, :],
                            w2rep_sb[:, :ng * D].rearrange(
                                "p (k f) -> p k f", f=D))
                        lg = lgp.tile([P, MAXG // P], mybir.dt.float32, tag="lg")
                        nc.vector.tensor_reduce(
                            lg[:, :ng], gb[:, :ng, :],
                            axis=mybir.AxisListType.X, op=mybir.AluOpType.add)
                        nc.vector.tensor_scalar_add(
                            lg[:, :ng], lg[:, :ng], b2b_sb[:, 0:1])
                        nc.sync.dma_start(
                            out_t[:, out_off:out_off + ng], lg[:, :ng])
                        idx_off += nt // 16
                        out_off += ng
                        gsel_off += nr * ng

                def emit_all():
                    atw_tiles.clear()
                    phase_a_b()
                    phase_b()

                if loop_n is None:
                    emit_all()
                else:
                    with tc.For_i(0, loop_n, 1):
                        emit_all()

    nc.compile()
    names = dict(
        embT=embT_t.name, embTb=embTb_t.name, w1cat=w1cat_t.name,
        w2rep=w2rep_t.name, b2b=b2b_t.name, ones=ones_t.name,
        gsel=gsel_t.name, idst=idst_t.name, srcf=srcf_t.name, out=out_t.name,
    )
    return nc, names


def _host_streams(src, dst, tiles, tot_idx, m_g):
    """Per-core idx/src streams + gsel table + output positions."""
    core = dst // B_NODES
    cell_start = np.zeros(N_GROUPS + 1, dtype=np.int64)
    np.cumsum(m_g, out=cell_start[1:])

    iota = np.arange(P, dtype=np.float32)
    gsel_cols = sum(t['nranks'] * 16 for t in tiles)
    gsel = np.full((P, gsel_cols), -2.0, dtype=np.float32)
    off = 0
    for t in tiles:
        ng = t['nt'] // P
        for (jl, r, g, st, sp) in t['visits']:
            if g is not None:
                gsel[:, off + r * ng + jl] = g * P + iota
        off += t['nranks'] * ng

    eorder, stream_pos, in_idx, in_src = [], [], [], []
    for c in range(N_CORES):
        ids = np.flatnonzero(core == c)
        ids = ids[np.argsort(src[ids], kind="stable")]
        eorder.append(ids)
        g_of = src[ids] // P
        gcounts = np.bincount(g_of, minlength=N_GROUPS)
        goff = np.concatenate([[0], np.cumsum(gcounts)])
        ranks = np.arange(len(ids)) - goff[g_of]
        pos = cell_start[g_of] + ranks
        stream_pos.append(pos)

        idst_s = np.zeros(tot_idx, dtype=np.int16)
        srcf_s = np.full(tot_idx, -1.0, dtype=np.float32)
        idst_s[pos] = (dst[ids] - c * B_NODES).astype(np.int16)
        srcf_s[pos] = src[ids].astype(np.float32)

        idxw = np.empty((P, tot_idx // 16), dtype=np.int16)
        t0 = 0
        for t in tiles:
            nt = t['nt']
            w = idst_s[t0:t0 + nt].reshape(nt // 16, 16).T
            idxw[:, t0 // 16:(t0 + nt) // 16] = np.tile(w, (8, 1))
            t0 += nt
        in_idx.append(idxw)
        in_src.append(srcf_s.reshape(1, -1))
    return gsel, eorder, stream_pos, in_idx, in_src


def kernel(nodes_emb, src, dst, W1, b1, W2, b2):
    from concourse.bass_utils import run_bass_kernel_spmd

    nodes_emb = np.ascontiguousarray(np.asarray(nodes_emb, dtype=np.float32))
    src = np.asarray(src).astype(np.int64)
    dst = np.asarray(dst).astype(np.int64)
    W1 = np.asarray(W1, dtype=np.float32)
    b1 = np.asarray(b1, dtype=np.float32).reshape(-1)
    W2 = np.asarray(W2, dtype=np.float32)
    b2 = np.asarray(b2, dtype=np.float32).reshape(-1)
    E = src.shape[0]

    embT = np.zeros((D + 1, R_PAD), dtype=np.float32)
    embT[:D, :N_NODES] = nodes_emb.T
    embT[D, :] = 1.0
    w1cat = np.zeros((D + 1, 2 * D), dtype=np.float32)
    w1cat[:D, :D] = W1[:D]
    w1cat[:D, D:] = W1[D:]
    w1cat[D, :D] = b1
    w2rep = np.tile(W2.reshape(1, D), (P, 1)).astype(np.float32)
    b2b = np.full((P, 1), b2[0], dtype=np.float32)
    ones = np.ones((1, P), dtype=np.float32)

    core = dst // B_NODES
    cnt = np.zeros((N_CORES, N_GROUPS), dtype=np.int64)
    for c in range(N_CORES):
        ids = np.flatnonzero(core == c)
        cnt[c] = np.bincount(src[ids] // P, minlength=N_GROUPS)
    m_g = cnt.max(axis=0)
    tot_idx, tiles = _build_schedule(m_g)

    global _last_sched
    _last_sched = (tiles, tot_idx, m_g)
    nc, names = _build_program(tiles, tot_idx)
    global _last_names
    _last_names = names

    gsel, eorder, stream_pos, in_idx, in_src = _host_streams(
        src, dst, tiles, tot_idx, m_g)

    in_maps = []
    for c in range(N_CORES):
        embTb = np.zeros((D + 1, B_PAD), dtype=np.float32)
        lo = c * B_NODES
        hi = min((c + 1) * B_NODES, N_NODES)
        embTb[:D, :hi - lo] = nodes_emb[lo:hi].T
        embTb[D, :] = 1.0
        in_maps.append({
            names["embT"]: embT,
            names["embTb"]: embTb,
            names["w1cat"]: w1cat,
            names["w2rep"]: w2rep,
            names["b2b"]: b2b,
            names["ones"]: ones,
            names["gsel"]: gsel,
            names["idst"]: in_idx[c],
            names["srcf"]: in_src[c],
        })

    global _last_results, _last_ctx
    res = run_bass_kernel_spmd(
        nc, in_maps, core_ids=list(range(N_CORES)), trace=TRACE,
    )
    _last_results = res
    _last_ctx = (nc, in_maps)

    out = np.empty(E, dtype=np.float32)
    for c in range(N_CORES):
        o = res.results[c][names["out"]]       # [128, tot_idx//128]
        pos = stream_pos[c]
        out[eorder[c]] = o[pos % P, pos // P]
    return out.reshape(E, 1)


# revision 3
# speedup vs baseline: 2.3910x; 1.0232x over previous
"""GNN edge-scorer for Trainium2 (8 NeuronCores, SPMD) — hybrid gather.

reference:
    edge_emb = concat(emb[src], emb[dst])          # [E, 128]
    h = relu(edge_emb @ W1 + b1)                   # [E, 64]
    logits = h @ W2 + b2                           # [E, 1]

Decomposition: A[n] = emb[n] @ W1[:64] + b1,  B[n] = emb[n] @ W1[64:]
    logit[e] = relu(A[src_e] + B[dst_e]) . w2 + b2

Distribution: core c owns dst-bucket [c*18752, (c+1)*18752); its edges are
sorted by src.  The per-core B-table (18944 rows f32, node-major in DRAM)
is computed on-device from a per-core embT slice; dst indices are
bucket-local int16 and one dma_gather source serves the whole run (no chunk
switching, 1 descriptor per edge instead of the baseline's 2).

The A-side uses no DMA gather at all.  Sorted-by-src edges form ~200-edge
runs per 128-node group ("cells").  Per 128-edge PSUM group, one matmul per
intersecting cell accumulates the A rows into PSUM: stationary = a one-hot
[node-low x edge] built on DVE (is_equal of PE-broadcast src ids against
per-group iota columns), moving = a 64-node-group bf16 A-table window that
phase A materializes directly in SBUF (no DRAM roundtrip), just-in-time in
source-group order.  Cell sizes are padded to the max across cores so one
SPMD program fits all cores; the host unscrambles the output.

Per-edge chain: h = relu(ps_A + gb) on DVE/ACT, then *w2rep, reduce, +b2.
"""

import numpy as np

N_NODES = 150000
TRACE = False
_last_results = None
_last_ctx = None
_last_names = None
_last_sched = None
D = 64
P = 128
N_CORES = 8
R_PAD = 150016                  # 293 * 512
B_NODES = R_PAD // N_CORES      # 18752 dst rows owned per core
B_PAD = 18944                   # 37 * 512
PRE_T = 512                     # phase-A nodes per tile
MAXG = 2048                     # edges per phase-B tile (16 PSUM col groups)
W_NODES = 8192                  # A-window nodes (64 groups of 128)
N_GROUPS = R_PAD // P           # 1172 src groups
QUEUES = 4


def _build_schedule(m_g):
    """Static tile schedule from per-src-group padded counts.

    Returns (tot_idx, tiles).  tiles: list of dicts
      nt       edges in tile (multiple of 128, <= MAXG)
      windows  A-windows to fill before this tile
      nranks   max visits over the tile's groups
      visits   [(j_local, r, g_or_None, start, stop)]
    """
    cells = [(g, int(m)) for g, m in enumerate(m_g) if m > 0]
    tot = int(sum(m for _, m in cells))
    tot_pad = -(-tot // P) * P
    if tot_pad > tot:
        cells.append((None, tot_pad - tot))
    n_groups_tot = tot_pad // P
    group_visits = [[] for _ in range(n_groups_tot)]
    pos = 0
    for g, m in cells:
        if g is not None:
            for j in range(pos // P, (pos + m - 1) // P + 1):
                group_visits[j].append(g)
        pos += m
    for gv in group_visits:
        if not gv:
            gv.append(None)

    tiles = []
    seen_w = set()
    j = 0
    while j < n_groups_tot:
        jn = min(16, n_groups_tot - j)
        t = {'nt': jn * P, 'windows': [], 'visits': [],
             'nranks': max(len(group_visits[j + jl]) for jl in range(jn))}
        for jl in range(jn):
            gv = group_visits[j + jl]
            for r, g in enumerate(gv):
                t['visits'].append((jl, r, g, r == 0, r == len(gv) - 1))
                if g is not None:
                    w = (g * P) // W_NODES
                    if w not in seen_w:
                        seen_w.add(w)
                        t['windows'].append(w)
        tiles.append(t)
        j += jn
    return tot_pad, tiles


def _build_program(tiles, tot_idx, loop_n=None, mode="full"):
    import concourse.bacc as bacc
    import concourse.mybir as mybir
    import concourse.tile as tile

    nr_max = max(t['nranks'] for t in tiles)
    gsel_cols = sum(t['nranks'] * 16 for t in tiles)

    nc = bacc.Bacc(None, target_bir_lowering=False, debug=False,
                   num_swdge_queues=QUEUES)
    with tile.TileContext(nc) as tc:
        with tc.tile_pool(name="dram", bufs=1, space="DRAM") as dram:
            embT_t = dram.tile([D + 1, R_PAD], mybir.dt.float32, kind="ExternalInput")
            embTb_t = dram.tile([D + 1, B_PAD], mybir.dt.float32, kind="ExternalInput")
            w1cat_t = dram.tile([D + 1, 2 * D], mybir.dt.float32, kind="ExternalInput")
            w2rep_t = dram.tile([P, D], mybir.dt.float32, kind="ExternalInput")
            b2b_t = dram.tile([P, 1], mybir.dt.float32, kind="ExternalInput")
            ones_t = dram.tile([1, P], mybir.dt.float32, kind="ExternalInput")
            gsel_t = dram.tile([P, gsel_cols], mybir.dt.float32, kind="ExternalInput")
            idst_t = dram.tile([P, tot_idx // 16], mybir.dt.int16, kind="ExternalInput")
            srcf_t = dram.tile([1, tot_idx], mybir.dt.float32, kind="ExternalInput")
            out_t = dram.tile([P, tot_idx // P], mybir.dt.float32, kind="ExternalOutput")
            bt_t = dram.tile([B_PAD, D], mybir.dt.float32, name="btab")

            with (
                tc.tile_pool(name="consts", bufs=1) as consts,
                tc.tile_pool(name="pre_et", bufs=3) as pre_et,
                tc.tile_pool(name="pre_st", bufs=3) as pre_st,
                tc.tile_pool(name="ps_sh", bufs=3, space="PSUM") as ps_sh,
                tc.tile_pool(name="atwp", bufs=3) as atwp,
                tc.tile_pool(name="idxp", bufs=3) as idxp,
                tc.tile_pool(name="srcp", bufs=3) as srcp,
                tc.tile_pool(name="gselp", bufs=3) as gselp,
                tc.tile_pool(name="gbp", bufs=3) as gbp,
                tc.tile_pool(name="ohp", bufs=2) as ohp,
                tc.tile_pool(name="ps_a", bufs=2, space="PSUM") as ps_ap,
                tc.tile_pool(name="lgp", bufs=3) as lgp,
            ):
                w1cat_sb = consts.tile([D + 1, 2 * D], mybir.dt.float32)
                nc.sync.dma_start(w1cat_sb[:], w1cat_t[:])
                b2b_sb = consts.tile([P, 1], mybir.dt.float32)
                nc.sync.dma_start(b2b_sb[:], b2b_t[:])
                ones_sb = consts.tile([1, P], mybir.dt.float32)
                nc.sync.dma_start(ones_sb[:], ones_t[:])
                zero64 = consts.tile([P, D], mybir.dt.bfloat16)
                nc.vector.memset(zero64[:], 0.0)
                w2in_sb = consts.tile([P, D], mybir.dt.float32, tag="w2in")
                nc.sync.dma_start(w2in_sb[:], w2rep_t[:])
                w2rep_sb = consts.tile([P, (MAXG // P) * D], mybir.dt.float32,
                                       tag="w2rep")
                for j in range(MAXG // P):
                    nc.vector.tensor_copy(w2rep_sb[:, j * D:(j + 1) * D],
                                          w2in_sb[:])

                def phase_a_b():
                    # per-core B table (node-major f32) -> bt_t
                    for i in range(B_PAD // PRE_T):
                        et = pre_et.tile([D + 1, PRE_T], mybir.dt.float32, tag="et")
                        nc.sync.dma_start(
                            et[:], embTb_t[:, i * PRE_T:(i + 1) * PRE_T])
                        nstg = pre_st.tile([P, PRE_T // P, D], mybir.dt.float32,
                                           tag="stgb")
                        for q in range(PRE_T // P):
                            ps_n = ps_sh.tile([P, D], mybir.dt.float32,
                                              space="PSUM")
                            nc.tensor.matmul(
                                ps_n[:], lhsT=et[:, q * P:(q + 1) * P],
                                rhs=w1cat_sb[:, D:2 * D], start=True, stop=True)
                            nc.scalar.activation(
                                nstg[:, q, :], ps_n[:],
                                mybir.ActivationFunctionType.Copy)
                        nc.sync.dma_start(
                            bt_t[i * PRE_T:(i + 1) * PRE_T, :].rearrange(
                                "(k p) f -> p k f", p=P),
                            nstg[:])

                atw_tiles = {}

                def fill_window(w):
                    # A-table window (bf16, node-major groups) built in SBUF
                    rows = min(W_NODES, R_PAD - w * W_NODES)
                    atw = atwp.tile([P, (W_NODES // P) * D], mybir.dt.bfloat16,
                                    tag="atw")
                    for i0 in range(0, rows, PRE_T):
                        base = w * W_NODES + i0
                        et = pre_et.tile([D + 1, PRE_T], mybir.dt.float32,
                                         tag="et")
                        nc.sync.dma_start(et[:], embT_t[:, base:base + PRE_T])
                        for q in range(PRE_T // P):
                            ps_n = ps_sh.tile([P, D], mybir.dt.float32,
                                              space="PSUM")
                            nc.tensor.matmul(
                                ps_n[:], lhsT=et[:, q * P:(q + 1) * P],
                                rhs=w1cat_sb[:, 0:D], start=True, stop=True)
                            col = ((i0 + q * P) // P) * D
                            nc.scalar.activation(
                                atw[:, col:col + D], ps_n[:],
                                mybir.ActivationFunctionType.Copy)
                    atw_tiles[w] = atw

                gq = [0]

                def phase_b():
                    idx_off = 0   # int16 idx columns consumed
                    out_off = 0
                    gsel_off = 0
                    for t in tiles:
                        nt = t['nt']
                        ng = nt // P
                        nr = t['nranks']
                        for w in t['windows']:
                            fill_window(w)
                        it = idxp.tile([P, MAXG // 16], mybir.dt.int16, tag="it")
                        nc.sync.dma_start(
                            it[:, :nt // 16],
                            idst_t[:, idx_off:idx_off + nt // 16])
                        gb = gbp.tile([P, MAXG // P, D], mybir.dt.float32,
                                      tag="gb")
                        if mode != "B_noB":
                            nc.gpsimd.dma_gather(
                                gb[:, :ng, :], bt_t[:, :],
                                it[:, :nt // 16], nt, nt, D, elem_step=D,
                                single_packet=False, queue_num=gq[0])
                            gq[0] = (gq[0] + 1) % QUEUES
                        if mode == "B_gather":
                            idx_off += nt // 16
                            out_off += ng
                            continue
                        use_a = mode != "B_noA"
                        if use_a:
                            sf = srcp.tile([1, MAXG], mybir.dt.float32, tag="sf")
                            nc.sync.dma_start(
                                sf[:, :nt],
                                srcf_t[:, idx_off * 16:idx_off * 16 + nt])
                            gs = gselp.tile([P, nr_max * 16], mybir.dt.float32,
                                            tag="gs")
                            nc.sync.dma_start(
                                gs[:, :nr * ng],
                                gsel_t[:, gsel_off:gsel_off + nr * ng])
                            oh = ohp.tile([P, nr_max * MAXG], mybir.dt.bfloat16,
                                          tag="oh")
                            for q0 in range(0, nt, 512):
                                qn = min(512, nt - q0)
                                ps_b = ps_sh.tile([P, 512], mybir.dt.float32,
                                                  space="PSUM")
                                nc.tensor.matmul(
                                    ps_b[:, :qn], lhsT=ones_sb[:],
                                    rhs=sf[:, q0:q0 + qn], start=True, stop=True)
                                for r in range(nr):
                                    nc.vector.tensor_tensor(
                                        oh[:, r * MAXG + q0:
                                           r * MAXG + q0 + qn].rearrange(
                                            "p (j e) -> p j e", e=P),
                                        ps_b[:, :qn].rearrange(
                                            "p (j e) -> p j e", e=P),
                                        gs[:, r * ng + q0 // P:
                                           r * ng + q0 // P + qn // P, None
                                           ].to_broadcast((P, qn // P, P)),
                                        mybir.AluOpType.is_equal)
                            ps_a = ps_ap.tile([P, (MAXG // P) * D],
                                              mybir.dt.float32, space="PSUM")
                            for (jl, r, g, st, sp) in t['visits']:
                                if g is None:
                                    rhs = zero64[:]
                                else:
                                    w = (g * P) // W_NODES
                                    o = ((g * P) % W_NODES // P) * D
                                    rhs = atw_tiles[w][:, o:o + D]
                                nc.tensor.matmul(
                                    ps_a[:, jl * D:(jl + 1) * D],
                                    lhsT=oh[:, r * MAXG + jl * P:
                                            r * MAXG + (jl + 1) * P],
                                    rhs=rhs, start=st, stop=sp)
                        # h = relu(A + B); logits = h . w2 + b2
                        if use_a and mode != "B_noB":
                            nc.vector.tensor_add(
                                gb[:, :ng, :],
                                ps_a[:, :ng * D].rearrange(
                                    "p (k f) -> p k f", f=D),
                                gb[:, :ng, :])
                        elif use_a:
                            nc.vector.tensor_copy(
                                gb[:, :ng, :],
                                ps_a[:, :ng * D].rearrange(
                                    "p (k f) -> p k f", f=D))
                        nc.scalar.activation(
                            gb[:, :ng, :], gb[:, :ng, :],
                            mybir.ActivationFunctionType.Relu)
                        nc.vector.tensor_mul(
                            gb[:, :ng, :], gb[:, :ng, :],
                            w2rep_sb[:, :ng * D].rearrange(
                                "p (k f) -> p k f", f=D))
                        lg = lgp.tile([P, MAXG // P], mybir.dt.float32, tag="lg")
                        nc.vector.tensor_reduce(
                            lg[:, :ng], gb[:, :ng, :],
                            axis=mybir.AxisListType.X, op=mybir.AluOpType.add)
                        nc.vector.tensor_scalar_add(
                            lg[:, :ng], lg[:, :ng], b2b_sb[:, 0:1])
                        nc.sync.dma_start(
                            out_t[:, out_off:out_off + ng], lg[:, :ng])
                        idx_off += nt // 16
                        out_off += ng
                        gsel_off += nr * ng

                def emit_all():
                    atw_tiles.clear()
                    phase_a_b()
                    phase_b()

                if loop_n is None:
                    emit_all()
                else:
                    with tc.For_i(0, loop_n, 1):
                        emit_all()

    nc.compile()
    names = dict(
        embT=embT_t.name, embTb=embTb_t.name, w1cat=w1cat_t.name,
        w2rep=w2rep_t.name, b2b=b2b_t.name, ones=ones_t.name,
        gsel=gsel_t.name, idst=idst_t.name, srcf=srcf_t.name, out=out_t.name,
    )
    return nc, names


def _host_streams(src, dst, tiles, tot_idx, m_g):
    """Per-core idx/src streams + gsel table + output positions."""
    core = dst // B_NODES
    cell_start = np.zeros(N_GROUPS + 1, dtype=np.int64)
    np.cumsum(m_g, out=cell_start[1:])

    iota = np.arange(P, dtype=np.float32)
    gsel_cols = sum(t['nranks'] * 16 for t in tiles)
    gsel = np.full((P, gsel_cols), -2.0, dtype=np.float32)
    off = 0
    for t in tiles:
        ng = t['nt'] // P
        for (jl, r, g, st, sp) in t['visits']:
            if g is not None:
                gsel[:, off + r * ng + jl] = g * P + iota
        off += t['nranks'] * ng

    eorder, stream_pos, in_idx, in_src = [], [], [], []
    for c in range(N_CORES):
        ids = np.flatnonzero(core == c)
        ids = ids[np.argsort(src[ids], kind="stable")]
        eorder.append(ids)
        g_of = src[ids] // P
        gcounts = np.bincount(g_of, minlength=N_GROUPS)
        goff = np.concatenate([[0], np.cumsum(gcounts)])
        ranks = np.arange(len(ids)) - goff[g_of]
        pos = cell_start[g_of] + ranks
        stream_pos.append(pos)

        idst_s = np.zeros(tot_idx, dtype=np.int16)
        srcf_s = np.full(tot_idx, -1.0, dtype=np.float32)
        idst_s[pos] = (dst[ids] - c * B_NODES).astype(np.int16)
        srcf_s[pos] = src[ids].astype(np.float32)

        idxw = np.empty((P, tot_idx // 16), dtype=np.int16)
        t0 = 0
        for t in tiles:
            nt = t['nt']
            w = idst_s[t0:t0 + nt].reshape(nt // 16, 16).T
            idxw[:, t0 // 16:(t0 + nt) // 16] = np.tile(w, (8, 1))
            t0 += nt
        in_idx.append(idxw)
        in_src.append(srcf_s.reshape(1, -1))
    return gsel, eorder, stream_pos, in_idx, in_src


def kernel(nodes_emb, src, dst, W1, b1, W2, b2):
    from concourse.bass_utils import run_bass_kernel_spmd

    nodes_emb = np.ascontiguousarray(np.asarray(nodes_emb, dtype=np.float32))
    src = np.asarray(src).astype(np.int64)
    dst = np.asarray(dst).astype(np.int64)
    W1 = np.asarray(W1, dtype=np.float32)
    b1 = np.asarray(b1, dtype=np.float32).reshape(-1)
    W2 = np.asarray(W2, dtype=np.float32)
    b2 = np.asarray(b2, dtype=np.float32).reshape(-1)
    E = src.shape[0]

    embT = np.zeros((D + 1, R_PAD), dtype=np.float32)
    embT[:D, :N_NODES] = nodes_emb.T
    embT[D, :] = 1.0
    w1cat = np.zeros((D + 1, 2 * D), dtype=np.float32)
    w1cat[:D, :D] = W1[:D]
    w1cat[:D, D:] = W1[D:]
    w1cat[D, :D] = b1
    w2rep = np.tile(W2.reshape(1, D), (P, 1)).astype(np.float32)
    b2b = np.full((P, 1), b2[0], dtype=np.float32)
    ones = np.ones((1, P), dtype=np.float32)

    core = dst // B_NODES
    cnt = np.zeros((N_CORES, N_GROUPS), dtype=np.int64)
    for c in range(N_CORES):
        ids = np.flatnonzero(core == c)
        cnt[c] = np.bincount(src[ids] // P, minlength=N_GROUPS)
    # round cells to 128-edge multiples: no cell straddles a PSUM group,
    # so each group needs exactly one one-hot matmul (nranks==1)
    m_g = (cnt.max(axis=0) + P - 1) // P * P
    tot_idx, tiles = _build_schedule(m_g)

    global _last_sched
    _last_sched = (tiles, tot_idx, m_g)
    nc, names = _build_program(tiles, tot_idx)
    global _last_names
    _last_names = names

    gsel, eorder, stream_pos, in_idx, in_src = _host_streams(
        src, dst, tiles, tot_idx, m_g)

    in_maps = []
    for c in range(N_CORES):
        embTb = np.zeros((D + 1, B_PAD), dtype=np.float32)
        lo = c * B_NODES
        hi = min((c + 1) * B_NODES, N_NODES)
        embTb[:D, :hi - lo] = nodes_emb[lo:hi].T
        embTb[D, :] = 1.0
        in_maps.append({
            names["embT"]: embT,
            names["embTb"]: embTb,
            names["w1cat"]: w1cat,
            names["w2rep"]: w2rep,
            names["b2b"]: b2b,
            names["ones"]: ones,
            names["gsel"]: gsel,
            names["idst"]: in_idx[c],
            names["srcf"]: in_src[c],
        })

    global _last_results, _last_ctx
    res = run_bass_kernel_spmd(
        nc, in_maps, core_ids=list(range(N_CORES)), trace=TRACE,
    )
    _last_results = res
    _last_ctx = (nc, in_maps)

    out = np.empty(E, dtype=np.float32)
    for c in range(N_CORES):
        o = res.results[c][names["out"]]       # [128, tot_idx//128]
        pos = stream_pos[c]
        out[eorder[c]] = o[pos % P, pos // P]
    return out.reshape(E, 1)
